# revision 10
# baseline (speedup 1.0000x reference)
"""AttentionPooling Trainium2 kernel (8 NeuronCores, Bass/Tile).

Sharding: (batch, head-group) — core c handles batch b=c//2 and heads
4*(c%2)..4*(c%2)+3. Each core computes, for its 4 heads, Q^T/K^T (head-dim
major) projections, then a one-pass pooled attention over 64 query stripes
(4 heads x 16 stripes of 128 queries):

  S = Q_stripe K^T / sqrt(d)      (PE, bf16, 4 matmuls into 4 PSUM banks)
  E = exp(S), Z = rowsum(E)       (ScalarE: ONE 2048-wide ACTIVATE + accum)
  r = 1/Z                         (VectorE)
  w_stripe = r^T E                (PE, 4 col-tiled matmuls into 1 bank)
  w_acc += w_stripe               (VectorE f32 add, PSUM->SBUF)

PSUM is managed as one manually-rotated [128, 8, 512] f32 ring:
  - stripe i's scores live in banks 4*(i%2)..+3, so each stripe's exp is a
    single contiguous 2048-wide ACTIVATE (one READ_ACCUMULATOR instead of
    two: ~0.5us/stripe saved on the ScalarE critical path).
  - the stripe's w matmuls reuse bank 4*(i%2) right after the ACTIVATE
    frees it (transient accumulator, k-chunk j at partition offset 32j),
    then VectorE folds it into an SBUF f32 accumulator.
  - interleaved Q/K projection chunks borrow the other 6 banks round-robin.

The V projection is never materialized: attended_mean*N = (w @ x) @ Wv_h^T
(+ bv folded on the host), so the tail computes u = w @ x (16 matmuls
against token-major x) and u @ Wv^T. The mean-pool is folded through the
output projection; V/output biases fold on the host:
  pooled = pooled_partial(core even) + pooled_partial(core odd) + Wo@bv + bo

Non-critical DMAs (remaining heads' Q/K weights, token-major x, Wv, Wo) are
dependency-gated on the first ACTIVATE so the prologue's HBM bandwidth all
goes to x^T + head-0 weights (first exp starts ~20us in instead of ~34us).
"""

import sys

import numpy as np

for _p in ("/opt/trn_rl_repo",):
    if _p not in sys.path:
        sys.path.append(_p)

import ml_dtypes

B, N, HID = 4, 2048, 1024
HEADS, HD = 8, 128
NH = 4          # heads per core
HGW = NH * HD   # head-group width (512)
NCORES = 8
P = 128
IT = HID // P   # 8 i-tiles
QT_TILES = N // P    # 16 query stripes
TOK_TILES = N // P   # 16 token tiles

BF16 = ml_dtypes.bfloat16

_cache = {}


def _build_nc():
    import concourse.bacc as bacc
    import concourse.tile as tile
    from concourse import mybir
    from concourse.bass import ds, ts
    from concourse.masks import make_identity
    from concourse.tile import add_dep_helper

    BF = mybir.dt.bfloat16
    F32 = mybir.dt.float32
    AF = mybir.ActivationFunctionType

    nc = bacc.Bacc(trn_type="TRN2")

    xT_d = nc.dram_tensor("xT", (HID, N), BF, kind="ExternalInput").ap()
    xtok_d = nc.dram_tensor("xtok", (N, HID), BF, kind="ExternalInput").ap()
    wqT_d = nc.dram_tensor("wqT", (NH, HID, HD), BF, kind="ExternalInput").ap()
    wkT_d = nc.dram_tensor("wkT", (NH, HID, HD), BF, kind="ExternalInput").ap()
    wvT_d = nc.dram_tensor("wvT", (HID, HGW), BF, kind="ExternalInput").ap()
    woT_d = nc.dram_tensor("woT", (HGW, HID), BF, kind="ExternalInput").ap()
    bq_d = nc.dram_tensor("bq_col", (P, NH), F32, kind="ExternalInput").ap()
    bk_d = nc.dram_tensor("bk_col", (P, NH), F32, kind="ExternalInput").ap()
    out_d = nc.dram_tensor("out_pooled", (1, HID), F32, kind="ExternalOutput").ap()

    inv_sqrt_d = float(1.0 / np.sqrt(HD))

    with tile.TileContext(nc) as tc:
        with (
            tc.tile_pool(name="persist", bufs=1) as persist,
            tc.tile_pool(name="ring", bufs=1, space="PSUM") as ringp,
            tc.tile_pool(name="ep", bufs=3) as ep,
            tc.tile_pool(name="zp", bufs=4) as zp,
        ):
            # ---- critical-path DMAs (everything else is gated on act0) ----
            xT_sb = persist.tile([P, IT, N], BF)
            wq_sb = persist.tile([P, IT, NH, HD], BF)
            wk_sb = persist.tile([P, IT, NH, HD], BF)
            xT_r = xT_d.rearrange("(t p) n -> p t n", p=P)
            wqT_r = wqT_d.rearrange("h (t p) d -> h p t d", p=P)
            wkT_r = wkT_d.rearrange("h (t p) d -> h p t d", p=P)
            bq_sb = persist.tile([P, NH], F32)
            bk_sb = persist.tile([P, NH], F32)
            nc.sync.dma_start(out=bq_sb, in_=bq_d)
            nc.sync.dma_start(out=bk_sb, in_=bk_d)
            nc.sync.dma_start(out=wk_sb[:, :, 0, :], in_=wkT_r[0])
            nc.sync.dma_start(out=wq_sb[:, :, 0, :], in_=wqT_r[0])
            # x^T in four 1MiB quarters: early quarters' projection matmuls
            # run while the later quarters transfer
            for qq in range(4):
                nc.sync.dma_start(
                    out=xT_sb[:, 2 * qq : 2 * qq + 2, :],
                    in_=xT_r[:, 2 * qq : 2 * qq + 2, :],
                )
            # tiles for the gated DMAs (emitted inside the stripe loop)
            xtok_sb = persist.tile([P, TOK_TILES, HID], BF)
            wv_sb = persist.tile([P, IT, HGW], BF)
            wo_sb = persist.tile([P, NH, HID], BF)

            ident = persist.tile([NH, NH], F32)
            make_identity(nc, ident)
            # one-hot columns: oneh_sb[p, h, h'] = 1.0 iff h == h'
            oneh_sb = persist.tile([P, NH, NH], BF)
            nc.vector.memset(oneh_sb, 0.0)
            for h in range(NH):
                nc.vector.memset(oneh_sb[:, h, h : h + 1], 1.0)

            QT_sb = persist.tile([P, NH, N], BF)
            KT_sb = persist.tile([P, NH, N], BF)
            # w accumulator, packed: k-chunk j on partitions 32j..32j+3
            w_acc = persist.tile([P, 512], F32)
            nc.vector.memset(w_acc, 0.0)
            w4_sb = persist.tile([NH, N], F32)
            # wT4[p, t, h] = w_h[t*128+p]  (token-major w for the u matmuls)
            wT4_sb = persist.tile([P, TOK_TILES, NH], BF)
            u4_sb = persist.tile([NH, HID], F32)
            # uTz[p, i, h, h'] = u_h[i*128+p] iff h' == h else 0 (block-diag
            # zero padding so per-head u@Wv^T matmuls share one accumulator)
            uTz_sb = persist.tile([P, IT, NH, NH], BF)
            nc.vector.memset(uTz_sb, 0.0)
            att4_sb = persist.tile([NH, P], F32)
            attT_sb = persist.tile([P, NH], BF)
            pooled_sb = persist.tile([1, HID], F32)

            # ---- the 8-bank PSUM ring ----
            R = ringp.tile([P, 8, 512], F32, name="ring")

            def qk_chunk(proj_i, h, c, ps):
                """One 512-token Q^T/K^T projection chunk for head h into the
                given PSUM bank AP (emitted atomically: the 8-matmul PSUM
                accumulation group must not interleave with other writers of
                the same bank in program order)."""
                wsb, bsb, dst = (
                    (wq_sb, bq_sb, QT_sb),
                    (wk_sb, bk_sb, KT_sb),
                )[proj_i]
                for i in range(IT):
                    nc.tensor.matmul(
                        ps,
                        lhsT=wsb[:, i, h, :],
                        rhs=xT_sb[:, i, ts(c, 512)],
                        start=(i == 0),
                        stop=(i == IT - 1),
                    )
                nc.vector.tensor_copy(dst[:, h, ts(c, 512)], ps)
                # per-partition bias (in-place, stride-0 free-dim broadcast)
                nc.vector.tensor_tensor(
                    dst[:, h, ts(c, 512)],
                    dst[:, h, ts(c, 512)],
                    bsb[:, h : h + 1].to_broadcast((P, 512)),
                    mybir.AluOpType.add,
                )

            # ---------------- prologue: head 0's K + first Q chunk --------
            # K chunks in banks 4..7 (stripe 1's group), Q chunk in bank 0:
            # stripe 0's score matmuls only serialize on the Q-chunk evac.
            for c in range(4):
                qk_chunk(1, 0, c, R[:, 4 + c, :])
            qk_chunk(0, 0, 0, R[:, 0, :])

            # Background projection work: remaining heads' Q/K chunks,
            # one whole chunk per eligible stripe. A chunk emitted at
            # bg_advance(i) goes to a bank of group i%2 (excluding that
            # group's w bank 4*(i%2)): in program order that bank was last
            # read by ACT(i) and is next score-written by emit_S(i+2), which
            # is emitted after bg_advance(i) — so the chunk's accumulation
            # group never interleaves with another writer, and its hardware
            # window (the ACT(i+1) span) comfortably fits the 8 matmuls.
            bg_n = [0]
            bg_specs = []
            for c in range(1, 4):
                bg_specs.append((0, 0, c))
            for h2 in range(1, NH):
                # K chunks first: head h2's stripes start at stripe 16*h2
                # and need ALL of K^T(h2) but only the first Q chunk.
                for c in range(4):
                    bg_specs.append((1, h2, c))
                for c in range(4):
                    bg_specs.append((0, h2, c))
            bg_specs.reverse()

            def bg_advance(si):
                # front-load: every stripe for the first 8, then every other
                if not bg_specs or (si >= 8 and si % 2):
                    return
                g = si % 2
                bank = 4 * g + 1 + bg_n[0] % 3
                bg_n[0] += 1
                qk_chunk(*bg_specs.pop(), R[:, bank, :])

            # ---------------- pooled attention stripe loop ----------------
            def emit_S(h, qi, grp):
                for kc in range(4):
                    nc.tensor.matmul(
                        R[:, 4 * grp + kc, :],
                        lhsT=QT_sb[:, h, ts(qi, P)],
                        rhs=KT_sb[:, h, ds(kc * 512, 512)],
                        start=True,
                        stop=True,
                    )

            NSTRIPES = NH * QT_TILES
            emit_S(0, 0, 0)
            for i in range(NSTRIPES):
                h, qi = i // QT_TILES, i % QT_TILES
                b0 = 4 * (i % 2)
                e_t = ep.tile([P, N], BF, tag="e", name="e_t")
                z_t = zp.tile([P, 1], F32, tag="z", name="z_t")
                act = nc.scalar.activation(
                    out=e_t,
                    in_=R[:, b0 : b0 + 4, :].rearrange("p a b -> p (a b)"),
                    func=AF.Exp,
                    scale=inv_sqrt_d,
                    accum_out=z_t,
                )
                if i == 0:
                    # non-critical DMAs, gated so they don't steal prologue
                    # HBM bandwidth from x^T / head-0 weights
                    gated = []
                    for h2 in range(1, NH):
                        gated.append(
                            nc.sync.dma_start(out=wk_sb[:, :, h2, :], in_=wkT_r[h2])
                        )
                        gated.append(
                            nc.sync.dma_start(out=wq_sb[:, :, h2, :], in_=wqT_r[h2])
                        )
                    gated.append(
                        nc.sync.dma_start(
                            out=xtok_sb,
                            in_=xtok_d.rearrange("(t p) d -> p t d", p=P),
                        )
                    )
                    gated.append(
                        nc.sync.dma_start(
                            out=wv_sb, in_=wvT_d.rearrange("(t p) d -> p t d", p=P)
                        )
                    )
                    gated.append(
                        nc.sync.dma_start(
                            out=wo_sb, in_=woT_d.rearrange("(t p) o -> p t o", p=P)
                        )
                    )
                    for g in gated:
                        add_dep_helper(g.ins, act.ins, sync=True, reason="defer-dma")
                if i + 1 < NSTRIPES:
                    ni = i + 1
                    emit_S(ni // QT_TILES, ni % QT_TILES, ni % 2)
                r_t = zp.tile([P, 1], F32, tag="r", name="r_t")
                nc.vector.reciprocal(r_t, z_t)
                # rb4 column h = r (bf16), other columns zero
                rb4_t = zp.tile([P, NH], BF, tag="rb", name="rb4_t")
                nc.vector.tensor_tensor(
                    rb4_t,
                    oneh_sb[:, h, :],
                    r_t.to_broadcast((P, NH)),
                    mybir.AluOpType.mult,
                )
                # transient w accumulation in the bank the ACTIVATE just read
                for j in range(4):
                    # each k-chunk region is written by exactly ONE matmul
                    # (own start/stop group): start=True's has_written clear
                    # must not let a sibling region accumulate stale scores
                    nc.tensor.matmul(
                        R[32 * j : 32 * j + NH, b0, :],
                        lhsT=rb4_t,
                        rhs=e_t[:, ts(j, 512)],
                        start=True,
                        stop=True,
                        tile_position=(0, 32 * j) if j else None,
                        skip_group_check=True,
                    )
                nc.vector.tensor_tensor(
                    w_acc, w_acc, R[:, b0, :], mybir.AluOpType.add
                )
                # interleaved background projection work
                bg_advance(i)

            # ---------------- tail ----------------
            # w_acc (packed f32) -> w4_sb [4, 2048] bf16
            for j in range(4):
                nc.vector.tensor_copy(w4_sb[:, ts(j, 512)], w_acc[32 * j : 32 * j + NH, :])

            # pipelined: transpose w4 chunk t -> wT4, then its two u matmuls
            # (u = w @ x accumulated in banks 2 and 3)
            for t in range(TOK_TILES):
                tpps = R[:, 5 + t % 3, 0:NH]  # [P, 4] f32
                nc.tensor.transpose(tpps, w4_sb[:, ts(t, P)], ident)
                nc.vector.tensor_copy(wT4_sb[:, t, :], tpps)
                for dc in range(2):
                    nc.tensor.matmul(
                        R[0:NH, 2 + dc, :],
                        lhsT=wT4_sb[:, t, :],
                        rhs=xtok_sb[:, t, ts(dc, 512)],
                        start=(t == 0),
                        stop=(t == TOK_TILES - 1),
                    )
            for dc in range(2):
                nc.vector.tensor_copy(u4_sb[:, ts(dc, 512)], R[0:NH, 2 + dc, :])
            # pipelined: transpose u chunk i -> uTz (block-diag scatter), then
            # its 4 att matmuls (att4 = u @ Wv^T accumulated in bank 1)
            for i in range(IT):
                tpps = R[:, 5 + i % 3, 0:NH]  # [P, 4] f32
                nc.tensor.transpose(tpps, u4_sb[:, ts(i, P)], ident)
                nc.vector.tensor_copy(
                    uTz_sb[:, i].rearrange("p a b -> p (a b)")[:, :: NH + 1],
                    tpps,
                )
                for h in range(NH):
                    nc.tensor.matmul(
                        R[0:NH, 1, 0:P],
                        lhsT=uTz_sb[:, i, h, :],
                        rhs=wv_sb[:, i, ts(h, HD)],
                        start=(i == 0 and h == 0),
                        stop=(i == IT - 1 and h == NH - 1),
                    )
            nc.vector.tensor_copy(att4_sb, R[0:NH, 1, 0:P])
            attT_ps = R[:, 0, 0:NH]
            nc.tensor.transpose(attT_ps, att4_sb, ident)
            nc.vector.tensor_copy(attT_sb, attT_ps)
            for oc in range(2):
                for h in range(NH):
                    nc.tensor.matmul(
                        R[0:1, 4 + oc, :],
                        lhsT=attT_sb[:, h : h + 1],
                        rhs=wo_sb[:, h, ts(oc, 512)],
                        start=(h == 0),
                        stop=(h == NH - 1),
                    )
            for oc in range(2):
                nc.vector.tensor_copy(pooled_sb[:, ts(oc, 512)], R[0:1, 4 + oc, :])
            nc.sync.dma_start(out=out_d, in_=pooled_sb)

    nc.finalize()  # Bacc: event-sem pass packs multi-waits into legal encodings
    return nc


def _get_nc():
    if "nc" not in _cache:
        _cache["nc"] = _build_nc()
    return _cache["nc"]


def _host_prep(inputs):
    """Build the 8 per-core input maps (host-side shard + transpose + cast)."""
    x = np.asarray(inputs["chunk_embeddings"], np.float32)
    in_maps = []
    for c in range(NCORES):
        b, hg = c // 2, c % 2
        sl = slice(hg * HGW, (hg + 1) * HGW)
        in_maps.append(
            {
                "xT": np.ascontiguousarray(x[b].T).astype(BF16),
                "xtok": np.ascontiguousarray(x[b]).astype(BF16),
                "wqT": np.ascontiguousarray(
                    np.asarray(inputs["Wq"], np.float32)[sl, :]
                    .T.reshape(HID, NH, HD)
                    .transpose(1, 0, 2)
                ).astype(BF16),
                "wkT": np.ascontiguousarray(
                    np.asarray(inputs["Wk"], np.float32)[sl, :]
                    .T.reshape(HID, NH, HD)
                    .transpose(1, 0, 2)
                ).astype(BF16),
                "wvT": np.ascontiguousarray(
                    np.asarray(inputs["Wv"], np.float32)[sl, :].T
                ).astype(BF16),
                "woT": np.ascontiguousarray(
                    np.asarray(inputs["Wo"], np.float32)[:, sl].T / np.float32(N)
                ).astype(BF16),
                "bq_col": np.ascontiguousarray(
                    np.asarray(inputs["bq"], np.float32)[sl].reshape(NH, P).T
                ),
                "bk_col": np.ascontiguousarray(
                    np.asarray(inputs["bk"], np.float32)[sl].reshape(NH, P).T
                ),
            }
        )
    return in_maps


def _unshard(results, inputs):
    bo = np.asarray(inputs["bo"], np.float32)
    bv = np.asarray(inputs["bv"], np.float32)
    Wo = np.asarray(inputs["Wo"], np.float32)
    bv_wo = Wo @ bv  # exact fold of the V bias through the output projection
    out = np.zeros((B, HID), np.float32)
    for b in range(B):
        out[b] = (
            results[2 * b]["out_pooled"][0]
            + results[2 * b + 1]["out_pooled"][0]
            + bv_wo
            + bo
        )
    return out


def _reference_numpy(inputs):
    """Fallback for non-trivial attention masks (never hit for the spec'd
    all-ones mask): straight numpy port of the reference."""
    x = np.asarray(inputs["chunk_embeddings"], np.float32)
    mask = np.asarray(inputs["attention_mask"])
    b, n, hid = x.shape

    def proj(W, bias):
        y = x @ np.asarray(W, np.float32).T + np.asarray(bias, np.float32)
        return y.reshape(b, n, HEADS, HD).transpose(0, 2, 1, 3)

    Q = proj(inputs["Wq"], inputs["bq"])
    K = proj(inputs["Wk"], inputs["bk"])
    V = proj(inputs["Wv"], inputs["bv"])
    s = np.einsum("bhqd,bhkd->bhqk", Q, K) / np.float32(np.sqrt(HD))
    s = np.where(mask[:, None, None, :] == 0, np.float32(-1e9), s)
    s = s - s.max(axis=-1, keepdims=True)
    e = np.exp(s)
    a = e / e.sum(axis=-1, keepdims=True)
    att = np.einsum("bhqk,bhkd->bhqd", a, V)
    att = att.transpose(0, 2, 1, 3).reshape(b, n, hid)
    out = att @ np.asarray(inputs["Wo"], np.float32).T + np.asarray(
        inputs["bo"], np.float32
    )
    m = mask[:, :, None].astype(np.float32)
    return (out * m).sum(axis=1) / m.sum(axis=1)


def _run(inputs, trace=False):
    from concourse.bass_utils import run_bass_kernel_spmd

    nc = _get_nc()
    in_maps = _host_prep(inputs)
    res = run_bass_kernel_spmd(
        nc, in_maps, core_ids=list(range(NCORES)), trace=trace
    )
    _cache["last_result"] = res
    return _unshard(res.results, inputs)


def kernel(**inputs):
    mask = np.asarray(inputs["attention_mask"])
    if not np.all(mask == 1):
        return _reference_numpy(inputs)
    return _run(inputs, trace=False)


def kernel_traced(**inputs):
    """Like kernel() but with NTFF profiling; returns (out, exec_time_ns)."""
    out = _run(inputs, trace=True)
    return out, _cache["last_result"].exec_time_ns
